# revision 16
# baseline (speedup 1.0000x reference)
"""GAT (2 heads x 256) + 4-layer MLP with LayerNorms + pairwise-distance output,
distributed over 8 Trainium2 NeuronCores via Bass/Tile.

Strategy (graph partition over destination nodes):
  - Stage A: each core computes its shard of the node table
    [h0 |1|0| h1 |1|0| a_src] = x @ [W_gat | v_src] (f32r-rounded), plus a
    local a_dst table; the node table is AllGather'ed.
  - GAT: edges pre-sorted by dst into blocks of 125 dsts (padded); per
    128-edge tile: indirect-DMA row gather; per-edge a_dst extracted by
    one-hot mask + reduce; w = exp(leaky_relu(a_src+a_dst, 0.2)); weighted
    one-hot ATw; U|z = ATw^T @ [h|1] on PE (softmax denominator via the
    embedded ones column); normalize by 1/z.
  - MLP fused per block (PE transpose + f32r matmul + LayerNorm on DVE),
    emitting transposed embeddings h3T [3, nodes/core]; AllGather.
  - cdist: d^2 = u.v with u=[h,1,sq_i], v=[-2h,sq_j,1]; both operands are
    staged as (-1/2)x scaled rows so each operand is produced by ONE
    uniform base-0 DVE scale op (partition-alignment rule); the epilogue
    computes sqrt(max(d2,0)). Diagonal zeroed exactly by
    an indirect-DMA scatter. Output row-sharded [N/8, N] per core.
"""
import sys
import os

for _p in ("/opt/trn_rl_repo", "/root/.axon_site/_ro/trn_rl_repo"):
    if os.path.isdir(_p) and _p not in sys.path:
        sys.path.append(_p)

import numpy as np
from concourse import bass, bacc, tile, mybir
from concourse.bass_utils import run_bass_kernel_spmd
from concourse.masks import make_identity

F32 = mybir.dt.float32
F32R = mybir.dt.float32r
I32 = mybir.dt.int32
AF = mybir.ActivationFunctionType
OP = mybir.AluOpType
AX = mybir.AxisListType

NC = 8
LN_EPS = 1e-5


class Cfg:
    def __init__(self, N=10000, EPB=4480, FD=512):
        self.N = N
        self.FD = FD
        self.NPC = N // NC
        self.BLK = 125
        self.NBLK = self.NPC // self.BLK
        self.EPB = EPB
        self.KT = EPB // 128
        self.NKT = self.NBLK * self.KT
        self.TW = 528
        assert self.NPC % self.BLK == 0 and EPB % 128 == 0
        npc = self.NPC
        c0 = (npc // 3 + 1) & ~1
        c1 = ((npc - c0) // 2 + 1) & ~1
        c2 = npc - c0 - c1
        self.CH = [c0, c1, c2]
        assert all(cw % 2 == 0 and cw <= 512 for cw in self.CH)
        assert sum(self.CH) == npc


def _ap(t_ap, offset, pattern):
    return bass.AP(t_ap.tensor, offset, pattern)


def build_nc(cfg: Cfg):
    c = cfg
    nc = bacc.Bacc("TRN2", target_bir_lowering=False, debug=False,
                   enable_asserts=True, num_devices=NC)
    xT = nc.dram_tensor("xT", [c.FD, c.NPC], F32, kind="ExternalInput").ap()
    offs = nc.dram_tensor("offs", [128, c.NKT], I32, kind="ExternalInput").ap()
    dstm = nc.dram_tensor("dstm", [128, c.NKT], F32, kind="ExternalInput").ap()
    iota = nc.dram_tensor("iota", [128, 128], F32, kind="ExternalInput").ap()
    wgat = nc.dram_tensor("wgat", [c.FD, 512], F32, kind="ExternalInput").ap()
    vsd = nc.dram_tensor("vsd", [c.FD, 4], F32, kind="ExternalInput").ap()
    wa = nc.dram_tensor("wa", [512, 256], F32, kind="ExternalInput").ap()
    w1 = nc.dram_tensor("w1", [256, 128], F32, kind="ExternalInput").ap()
    w2 = nc.dram_tensor("w2", [128, 64], F32, kind="ExternalInput").ap()
    w3 = nc.dram_tensor("w3", [64, 3], F32, kind="ExternalInput").ap()
    bgat = nc.dram_tensor("bgat", [128, 512], F32, kind="ExternalInput").ap()
    ba_r = nc.dram_tensor("ba_r", [128, 256], F32, kind="ExternalInput").ap()
    ga_r = nc.dram_tensor("ga_r", [128, 256], F32, kind="ExternalInput").ap()
    bna_r = nc.dram_tensor("bna_r", [128, 256], F32, kind="ExternalInput").ap()
    b1_r = nc.dram_tensor("b1_r", [128, 128], F32, kind="ExternalInput").ap()
    g1_r = nc.dram_tensor("g1_r", [128, 128], F32, kind="ExternalInput").ap()
    bn1_r = nc.dram_tensor("bn1_r", [128, 128], F32, kind="ExternalInput").ap()
    b2_r = nc.dram_tensor("b2_r", [128, 64], F32, kind="ExternalInput").ap()
    g2_r = nc.dram_tensor("g2_r", [128, 64], F32, kind="ExternalInput").ap()
    bn2_r = nc.dram_tensor("bn2_r", [128, 64], F32, kind="ExternalInput").ap()
    b3c = nc.dram_tensor("b3c", [3, 1], F32, kind="ExternalInput").ap()
    dgo = nc.dram_tensor("dgo", [128, 16], I32, kind="ExternalInput").ap()
    constm05 = nc.dram_tensor("constm05", [1, c.N], F32,
                              kind="ExternalInput").ap()
    D = nc.dram_tensor("D", [c.NPC, c.N], F32, kind="ExternalOutput").ap()
    DBG = os.environ.get("KDBG") == "1"
    if DBG:
        dbg_h3 = nc.dram_tensor("dbg_h3", [3 * NC, c.NPC], F32,
                                kind="ExternalOutput").ap()
        dbg_tbl = nc.dram_tensor("dbg_tbl", [128, c.TW], F32,
                                 kind="ExternalOutput").ap()
        dbg_u = nc.dram_tensor("dbg_u", [125, 516], F32,
                               kind="ExternalOutput").ap()
        dbg_ad = nc.dram_tensor("dbg_ad", [128, 8], F32,
                                kind="ExternalOutput").ap()
    tbl_sh = nc.dram_tensor("tbl_sh", [c.NPC, c.TW], F32, kind="Internal").ap()
    tbl_full = nc.dram_tensor("tbl_full", [c.N, c.TW], F32,
                              kind="Internal", addr_space="Shared").ap()
    attnD = nc.dram_tensor("attnD", [c.NPC, 2], F32, kind="Internal").ap()
    h3T_sh = nc.dram_tensor("h3T_sh", [3, c.NPC], F32, kind="Internal").ap()
    h3T_full = nc.dram_tensor("h3T_full", [3 * NC, c.NPC], F32,
                              kind="Internal", addr_space="Shared").ap()
    sqd = nc.dram_tensor("sqd", [1, c.N], F32, kind="Internal").ap()
    usqd = nc.dram_tensor("usqd", [1, c.NPC], F32, kind="Internal").ap()

    def load_w_r(pool, apx, K, Nn, tag):
        tiles = []
        for k in range(0, K, 128):
            tf = pool.tile([128, Nn], F32, tag=f"{tag}f")
            nc.sync.dma_start(out=tf[:], in_=apx[k:k + 128, :])
            tr = pool.tile([128, Nn], F32R, tag=f"{tag}r{k}")
            nc.vector.tensor_copy(tr[:], tf[:])
            tiles.append(tr)
        return tiles

    with tile.TileContext(nc) as tc:
        with tc.tile_pool(name="const", bufs=1) as cp:
            # ---------- constants ----------
            iota_sb = cp.tile([128, 128], F32)
            nc.sync.dma_start(out=iota_sb[:], in_=iota[:])
            ident = cp.tile([128, 128], F32)
            make_identity(nc, ident[:])
            c10 = cp.tile([128, 2], F32)
            nc.vector.memset(c10[:, 0:1], 1.0)
            nc.vector.memset(c10[:, 1:2], 0.0)
            zpad = cp.tile([128, 12], F32)
            nc.vector.memset(zpad[:], 0.0)
            ones31 = cp.tile([3, 1], F32)
            nc.vector.memset(ones31[:], 1.0)
            eps_sb = cp.tile([128, 1], F32)
            nc.vector.memset(eps_sb[:], LN_EPS)
            b3_sb = cp.tile([3, 1], F32)
            nc.sync.dma_start(out=b3_sb[:], in_=b3c[:])
            dgo_sb = cp.tile([128, 16], I32)
            nc.sync.dma_start(out=dgo_sb[:], in_=dgo[:])
            reps = {}
            for nm, apx, wdt in (("bgat", bgat, 512), ("ba", ba_r, 256),
                                 ("ga", ga_r, 256), ("bna", bna_r, 256),
                                 ("b1", b1_r, 128), ("g1", g1_r, 128),
                                 ("bn1", bn1_r, 128), ("b2", b2_r, 64),
                                 ("g2", g2_r, 64), ("bn2", bn2_r, 64)):
                tl = cp.tile([128, wdt], F32, tag=f"rep_{nm}")
                nc.sync.dma_start(out=tl[:], in_=apx[:])
                reps[nm] = tl
            wa_t = load_w_r(cp, wa, 512, 256, "wa")
            w1_t = load_w_r(cp, w1, 256, 128, "w1")
            w2_t = load_w_r(cp, w2, 128, 64, "w2")
            w3_f = cp.tile([64, 3], F32)
            nc.sync.dma_start(out=w3_f[:], in_=w3[:])
            offs_sb = cp.tile([128, c.NKT], I32)
            nc.sync.dma_start(out=offs_sb[:], in_=offs[:])
            dstm_sb = cp.tile([128, c.NKT], F32)
            nc.sync.dma_start(out=dstm_sb[:], in_=dstm[:])
            h3T_sb = cp.tile([3, c.NPC], F32)

            # ---------- stage A ----------
            with tc.tile_pool(name="sa", bufs=2) as sa, \
                 tc.tile_pool(name="psa", bufs=2, space="PSUM") as psa:
                nk = c.FD // 128
                xT_r = []
                for k in range(nk):
                    xf = sa.tile([128, c.NPC], F32, tag="xf")
                    nc.sync.dma_start(out=xf[:], in_=xT[k * 128:(k + 1) * 128, :])
                    xr = sa.tile([128, c.NPC], F32R, tag=f"xr{k}")
                    nc.vector.tensor_copy(xr[:], xf[:])
                    xT_r.append(xr)
                wg_t = load_w_r(sa, wgat, c.FD, 512, "wg")
                vs_t = load_w_r(sa, vsd, c.FD, 4, "vs")
                for mt in range(c.NBLK):
                    ms = mt * 125
                    ph = psa.tile([125, 512], F32, space="PSUM", tag="ph")
                    pa = psa.tile([125, 4], F32, space="PSUM", tag="pa")
                    for k in range(nk):
                        lhs = xT_r[k][:, ms:ms + 125]
                        nc.tensor.matmul(ph[:], lhsT=lhs, rhs=wg_t[k][:],
                                         start=(k == 0), stop=(k == nk - 1))
                        nc.tensor.matmul(pa[:], lhsT=lhs, rhs=vs_t[k][:],
                                         start=(k == 0), stop=(k == nk - 1))
                    tb = sa.tile([125, c.TW], F32R, tag="tb")
                    nc.vector.tensor_copy(tb[:, 0:256], ph[:, 0:256])
                    nc.vector.tensor_copy(tb[:, 256:258], c10[0:125, :])
                    nc.vector.tensor_copy(tb[:, 258:514], ph[:, 256:512])
                    nc.vector.tensor_copy(tb[:, 514:516], c10[0:125, :])
                    nc.vector.tensor_copy(tb[:, 516:518], pa[:, 0:2])
                    nc.vector.tensor_copy(tb[:, 518:528], zpad[0:125, 0:10])
                    nc.sync.dma_start(out=tbl_sh[ms:ms + 125, :],
                                      in_=tb[:].bitcast(F32))
                    ad = sa.tile([125, 2], F32, tag="ad")
                    nc.vector.tensor_copy(ad[:], pa[:, 2:4])
                    nc.sync.dma_start(out=attnD[ms:ms + 125, :], in_=ad[:])

            nc.gpsimd.collective_compute(
                "AllGather", OP.bypass, ins=[tbl_sh[:]], outs=[tbl_full[:]],
                replica_groups=[list(range(NC))])
            tbl_full_r = tbl_full.bitcast(F32R)

            # ---------- GAT + MLP ----------
            with tc.tile_pool(name="gb", bufs=10) as gb, \
                 tc.tile_pool(name="eb", bufs=6) as eb, \
                 tc.tile_pool(name="mb", bufs=3) as mb, \
                 tc.tile_pool(name="psu", bufs=2, space="PSUM") as psu, \
                 tc.tile_pool(name="pst", bufs=2, space="PSUM") as pst:
                for b in range(c.NBLK):
                    adb = eb.tile([128, 250], F32, tag="adb")
                    for hh in range(2):
                        src_ap = _ap(attnD, 125 * b * 2 + hh,
                                     [[0, 128], [2, 125]])
                        nc.sync.dma_start(out=adb[:, hh * 125:(hh + 1) * 125],
                                          in_=src_ap)
                    U0 = psu.tile([125, 258], F32, space="PSUM", tag="U0")
                    U1 = psu.tile([125, 258], F32, space="PSUM", tag="U1")
                    for t in range(c.KT):
                        j = b * c.KT + t
                        G = gb.tile([128, c.TW], F32R, tag="G")
                        nc.gpsimd.indirect_dma_start(
                            out=G[:], out_offset=None, in_=tbl_full_r,
                            in_offset=bass.IndirectOffsetOnAxis(
                                ap=offs_sb[:, j:j + 1], axis=0))
                        ind01 = eb.tile([128, 125], F32, tag="ind01")
                        nc.vector.tensor_scalar(
                            out=ind01[:], in0=iota_sb[:, 0:125],
                            scalar1=dstm_sb[:, j:j + 1], scalar2=None,
                            op0=OP.is_equal)
                        msk = eb.tile([128, 250], F32, tag="msk")
                        i01 = ind01[:]
                        ind2 = _ap(i01, i01.offset,
                                   [i01.ap[0], [0, 2]] + list(i01.ap[1:]))
                        nc.vector.tensor_tensor(out=msk[:], in0=adb[:],
                                                in1=ind2, op=OP.mult)
                        ad_e = eb.tile([128, 2], F32, tag="ad_e")
                        mk = msk[:]
                        mk3 = _ap(mk, mk.offset, [mk.ap[0], [125, 2], [1, 125]])
                        nc.vector.reduce_sum(out=ad_e[:], in_=mk3, axis=AX.X)
                        l_e = eb.tile([128, 2], F32, tag="l_e")
                        nc.vector.tensor_tensor(
                            out=l_e[:], in0=ad_e[:],
                            in1=G[:, 516:518].bitcast(F32), op=OP.add)
                        lk = eb.tile([128, 2], F32, tag="lk")
                        nc.scalar.activation(lk[:], l_e[:], AF.Prelu, alpha=0.2)
                        w_e = eb.tile([128, 2], F32, tag="w_e")
                        nc.scalar.activation(w_e[:], lk[:], AF.Exp)
                        atw0 = eb.tile([128, 125], F32R, tag="atw0")
                        nc.vector.tensor_scalar(
                            out=atw0[:], in0=iota_sb[:, 0:125],
                            scalar1=dstm_sb[:, j:j + 1], scalar2=w_e[:, 0:1],
                            op0=OP.is_equal, op1=OP.mult)
                        atw1 = eb.tile([128, 125], F32R, tag="atw1")
                        nc.vector.tensor_scalar(
                            out=atw1[:], in0=iota_sb[:, 0:125],
                            scalar1=dstm_sb[:, j:j + 1], scalar2=w_e[:, 1:2],
                            op0=OP.is_equal, op1=OP.mult)
                        if DBG and b == 0 and t == 0:
                            dad = eb.tile([128, 8], F32, tag="dbgad")
                            nc.vector.tensor_copy(dad[:, 0:2], ad_e[:])
                            nc.vector.tensor_copy(dad[:, 2:4], l_e[:])
                            nc.vector.tensor_copy(dad[:, 4:6], w_e[:])
                            nc.vector.tensor_copy(
                                dad[:, 6:8], G[:, 516:518].bitcast(F32))
                            nc.sync.dma_start(out=dbg_ad[:], in_=dad[:])
                        st, sp = (t == 0), (t == c.KT - 1)
                        nc.tensor.matmul(U0[:], lhsT=atw0[:], rhs=G[:, 0:258],
                                         start=st, stop=sp)
                        nc.tensor.matmul(U1[:], lhsT=atw1[:], rhs=G[:, 258:516],
                                         start=st, stop=sp)
                    if DBG and b == 0:
                        du = mb.tile([125, 516], F32, tag="dbgu")
                        nc.vector.tensor_copy(du[:, 0:258], U0[:])
                        nc.vector.tensor_copy(du[:, 258:516], U1[:])
                        nc.sync.dma_start(out=dbg_u[:], in_=du[:])
                    rz = eb.tile([125, 2], F32, tag="rz")
                    nc.vector.reciprocal(rz[:, 0:1], U0[:, 256:257])
                    nc.vector.reciprocal(rz[:, 1:2], U1[:, 256:257])
                    a0 = mb.tile([125, 512], F32, tag="a0")
                    nc.vector.tensor_scalar(out=a0[:, 0:256], in0=U0[:, 0:256],
                                            scalar1=rz[:, 0:1], scalar2=None,
                                            op0=OP.mult)
                    nc.vector.tensor_scalar(out=a0[:, 256:512], in0=U1[:, 0:256],
                                            scalar1=rz[:, 1:2], scalar2=None,
                                            op0=OP.mult)
                    nc.vector.tensor_tensor(out=a0[:], in0=a0[:],
                                            in1=reps["bgat"][0:125, :], op=OP.add)
                    nc.vector.tensor_scalar(out=a0[:], in0=a0[:], scalar1=0.0,
                                            scalar2=None, op0=OP.max)

                    def dense(act, K, Nn, wt):
                        ps = pst.tile([125, 256], F32, space="PSUM", tag="mm")
                        nkk = K // 128
                        for kk in range(nkk):
                            tp = pst.tile([128, 125], F32, space="PSUM",
                                          tag="tp")
                            nc.tensor.transpose(
                                out=tp[:], in_=act[:, kk * 128:(kk + 1) * 128],
                                identity=ident[0:125, 0:125])
                            aT = mb.tile([128, 125], F32R, tag="aT")
                            nc.vector.tensor_copy(aT[:], tp[:])
                            nc.tensor.matmul(ps[:, 0:Nn], lhsT=aT[:],
                                             rhs=wt[kk][:],
                                             start=(kk == 0), stop=(kk == nkk - 1))
                        return ps

                    def ln(ps, Nn, bias, gam, bet, tag):
                        t_ = mb.tile([125, Nn], F32, tag=f"t{tag}")
                        nc.vector.tensor_tensor(out=t_[:], in0=ps[:, 0:Nn],
                                                in1=bias[0:125, :], op=OP.add)
                        s_ = eb.tile([125, 1], F32, tag=f"s{tag}")
                        nc.vector.reduce_sum(out=s_[:], in_=t_[:], axis=AX.X)
                        nc.vector.tensor_scalar(out=s_[:], in0=s_[:],
                                                scalar1=1.0 / Nn, scalar2=None,
                                                op0=OP.mult)
                        xc = mb.tile([125, Nn], F32, tag=f"xc{tag}")
                        nc.vector.tensor_scalar(out=xc[:], in0=t_[:],
                                                scalar1=s_[:], scalar2=None,
                                                op0=OP.subtract)
                        x2 = mb.tile([125, Nn], F32, tag=f"x2{tag}")
                        nc.vector.tensor_tensor(out=x2[:], in0=xc[:], in1=xc[:],
                                                op=OP.mult)
                        ss = eb.tile([125, 1], F32, tag=f"ss{tag}")
                        nc.vector.reduce_sum(out=ss[:], in_=x2[:], axis=AX.X)
                        sd = eb.tile([125, 1], F32, tag=f"sd{tag}")
                        nc.scalar.activation(sd[:], ss[:], AF.Sqrt,
                                             bias=eps_sb[0:125, 0:1],
                                             scale=1.0 / Nn)
                        rstd = eb.tile([125, 1], F32, tag=f"rstd{tag}")
                        nc.vector.reciprocal(rstd[:], sd[:])
                        y = mb.tile([125, Nn], F32, tag=f"y{tag}")
                        nc.vector.tensor_scalar(out=y[:], in0=xc[:],
                                                scalar1=rstd[:], scalar2=None,
                                                op0=OP.mult)
                        nc.vector.tensor_tensor(out=y[:], in0=y[:],
                                                in1=gam[0:125, :], op=OP.mult)
                        nc.vector.tensor_tensor(out=y[:], in0=y[:],
                                                in1=bet[0:125, :], op=OP.add)
                        return y

                    psa_ = dense(a0, 512, 256, wa_t)
                    ya = ln(psa_, 256, reps["ba"], reps["ga"], reps["bna"], "a")
                    nc.vector.tensor_scalar(out=ya[:], in0=ya[:], scalar1=0.0,
                                            scalar2=None, op0=OP.max)
                    ps1 = dense(ya, 256, 128, w1_t)
                    y1 = ln(ps1, 128, reps["b1"], reps["g1"], reps["bn1"], "b")
                    nc.vector.tensor_scalar(out=y1[:], in0=y1[:], scalar1=0.0,
                                            scalar2=None, op0=OP.max)
                    nc.scalar.activation(y1[:], y1[:], AF.Tanh)
                    ps2 = dense(y1, 128, 64, w2_t)
                    y2 = ln(ps2, 64, reps["b2"], reps["g2"], reps["bn2"], "c")
                    nc.vector.tensor_scalar(out=y2[:], in0=y2[:], scalar1=0.0,
                                            scalar2=None, op0=OP.max)
                    tp3 = pst.tile([128, 125], F32, space="PSUM", tag="tp")
                    nc.tensor.transpose(out=tp3[0:64, :], in_=y2[:],
                                        identity=ident[0:125, 0:125])
                    a3T = mb.tile([64, 125], F32, tag="a3T")
                    nc.vector.tensor_copy(a3T[:], tp3[0:64, :])
                    ph3 = pst.tile([125, 256], F32, space="PSUM", tag="mm")
                    nc.tensor.matmul(ph3[0:3, 0:125], lhsT=w3_f[:], rhs=a3T[:],
                                     start=True, stop=True)
                    nc.vector.tensor_scalar(out=h3T_sb[:, b * 125:(b + 1) * 125],
                                            in0=ph3[0:3, 0:125],
                                            scalar1=b3_sb[:, 0:1],
                                            scalar2=None, op0=OP.add)
            nc.sync.dma_start(out=h3T_sh[:], in_=h3T_sb[:])

            nc.gpsimd.collective_compute(
                "AllGather", OP.bypass, ins=[h3T_sh[:]], outs=[h3T_full[:]],
                replica_groups=[list(range(NC))])

            if DBG:
                dtmp = cp.tile([24, c.NPC], F32, tag="dbg1")
                nc.sync.dma_start(out=dtmp[:], in_=h3T_full[:])
                nc.sync.dma_start(out=dbg_h3[:], in_=dtmp[:])
                dtb = cp.tile([128, c.TW], F32, tag="dbg2")
                nc.sync.dma_start(out=dtb[:], in_=tbl_full[0:128, :])
                nc.sync.dma_start(out=dbg_tbl[:], in_=dtb[:])
            # ---------- cdist ----------
            with tc.tile_pool(name="cdb", bufs=1) as cdb, \
                 tc.tile_pool(name="ob", bufs=(1 if DBG else 2)) as ob, \
                 tc.tile_pool(name="psq", bufs=2, space="PSUM") as psq, \
                 tc.tile_pool(name="psd", bufs=6, space="PSUM") as psd:
                Vr = cdb.tile([5, c.N], F32R)
                Ur = cdb.tile([5, c.NPC], F32R)
                with tc.tile_pool(name="cbu", bufs=1) as cbu:
                    # Vf5 = [h | -sq_j/2 | -1/2] ; Vr = -2*Vf5 (f32r)
                    Vf5 = cbu.tile([5, c.N], F32, tag="Vf5")
                    for r in range(NC):
                        nc.sync.dma_start(
                            out=Vf5[0:3, r * c.NPC:(r + 1) * c.NPC],
                            in_=h3T_full[3 * r:3 * r + 3, :])
                    # sq of the f32r-rounded h (same bits the matmul sees)
                    otmp = cbu.tile([3, c.NPC], F32, tag="otmp")
                    hreg = cbu.tile([3, c.NPC], F32R, tag="hreg")
                    for r in range(NC):
                        base = r * c.NPC
                        nc.vector.tensor_copy(hreg[:],
                                              Vf5[0:3, base:base + c.NPC])
                        hsl = hreg[:].bitcast(F32)
                        nc.vector.tensor_tensor(out=otmp[:], in0=hsl,
                                                in1=hsl, op=OP.mult)
                        for ci, cw in enumerate(c.CH):
                            off = sum(c.CH[:ci])
                            pq = psq.tile([1, 512], F32, space="PSUM", tag="pq")
                            nc.tensor.matmul(pq[:, 0:cw], lhsT=ones31[:],
                                             rhs=otmp[:, off:off + cw],
                                             start=True, stop=True)
                            sqst = cbu.tile([1, 512], F32, tag="sqst")
                            nc.vector.tensor_scalar(
                                out=sqst[:, 0:cw], in0=pq[0:1, 0:cw],
                                scalar1=-0.5, scalar2=None, op0=OP.mult)
                            nc.sync.dma_start(
                                out=sqd[:, base + off:base + off + cw],
                                in_=sqst[:, 0:cw])
                    nc.sync.dma_start(out=Vf5[3:4, :], in_=sqd[:])
                    nc.sync.dma_start(out=Vf5[4:5, :], in_=constm05[:])
                    nc.vector.tensor_scalar(out=Vr[:], in0=Vf5[:],
                                            scalar1=-2.0, scalar2=None,
                                            op0=OP.mult)
                    # Uf5 = [-h/2 | -1/2 | -sq_i/2] ; Ur = -2*Uf5 (f32r)
                    Uf5 = cbu.tile([5, c.NPC], F32, tag="Uf5")
                    nc.sync.dma_start(out=Uf5[0:3, :], in_=h3T_sh[:])
                    nc.vector.tensor_scalar(out=Uf5[0:3, :], in0=Uf5[0:3, :],
                                            scalar1=-0.5, scalar2=None,
                                            op0=OP.mult)
                    nc.sync.dma_start(out=Uf5[3:4, :],
                                      in_=constm05[:, 0:c.NPC])
                    # own sq from the rounded own h (same bits as V rows)
                    ur3 = cbu.tile([3, c.NPC], F32R, tag="ur3")
                    nc.vector.tensor_scalar(out=ur3[:], in0=Uf5[0:3, :],
                                            scalar1=-2.0, scalar2=None,
                                            op0=OP.mult)
                    u3f = ur3[:].bitcast(F32)
                    nc.vector.tensor_tensor(out=otmp[:], in0=u3f, in1=u3f,
                                            op=OP.mult)
                    for ci, cw in enumerate(c.CH):
                        off = sum(c.CH[:ci])
                        pq = psq.tile([1, 512], F32, space="PSUM", tag="pq")
                        nc.tensor.matmul(pq[:, 0:cw], lhsT=ones31[:],
                                         rhs=otmp[:, off:off + cw],
                                         start=True, stop=True)
                        sqst = cbu.tile([1, 512], F32, tag="sqst")
                        nc.vector.tensor_scalar(
                            out=sqst[:, 0:cw], in0=pq[0:1, 0:cw],
                            scalar1=-0.5, scalar2=None, op0=OP.mult)
                        nc.sync.dma_start(out=usqd[:, off:off + cw],
                                          in_=sqst[:, 0:cw])
                    nc.sync.dma_start(out=Uf5[4:5, :], in_=usqd[:])
                    nc.vector.tensor_scalar(out=Ur[:], in0=Uf5[:],
                                            scalar1=-2.0, scalar2=None,
                                            op0=OP.mult)
                for mt in range(c.NBLK):
                    ms = mt * 125
                    outb = ob.tile([125, c.N], F32, tag="outb")
                    for r in range(NC):
                        base = r * c.NPC
                        for ci, cw in enumerate(c.CH):
                            off = base + sum(c.CH[:ci])
                            pd = psd.tile([125, 512], F32, space="PSUM",
                                          tag="pd")
                            nc.tensor.matmul(pd[:, 0:cw],
                                             lhsT=Ur[:, ms:ms + 125],
                                             rhs=Vr[:, off:off + cw],
                                             start=True, stop=True)
                            nc.vector.tensor_scalar(
                                out=outb[:, off:off + cw], in0=pd[:, 0:cw],
                                scalar1=0.0, scalar2=None, op0=OP.max)
                            nc.scalar.activation(outb[:, off:off + cw],
                                                 outb[:, off:off + cw],
                                                 AF.Sqrt)
                    nc.sync.dma_start(out=D[ms:ms + 125, :], in_=outb[:])
                    nc.gpsimd.indirect_dma_start(
                        out=D[:], in_=c10[0:125, 1:2],
                        out_offset=bass.IndirectOffsetOnAxis(
                            ap=dgo_sb[0:125, mt:mt + 1], axis=1),
                        in_offset=None)
    nc.compile()
    return nc


def host_prep(cfg: Cfg, x, edge_index, W_gat, att_src, att_dst, b_gat,
              Wa, ba, ga, bna, W1, b1, g1, bn1, W2, b2, g2, bn2, W3, b3):
    c = cfg
    N = c.N
    x = np.asarray(x, np.float32)
    ei = np.asarray(edge_index).astype(np.int64)
    src = np.concatenate([ei[0], np.arange(N, dtype=np.int64)])
    dst = np.concatenate([ei[1], np.arange(N, dtype=np.int64)])
    order = np.argsort(dst, kind="stable")
    src, dst = src[order], dst[order]
    Wg = np.asarray(W_gat, np.float32)
    a_s = np.asarray(att_src, np.float32)
    a_d = np.asarray(att_dst, np.float32)
    Wg3 = Wg.reshape(c.FD, 2, 256)
    v_src = np.einsum("khc,hc->kh", Wg3, a_s).astype(np.float32)
    v_dst = np.einsum("khc,hc->kh", Wg3, a_d).astype(np.float32)
    vsd = np.concatenate([v_src, v_dst], axis=1)
    iota = np.tile(np.arange(128, dtype=np.float32), (128, 1))
    rep = lambda v, w: np.tile(np.asarray(v, np.float32).reshape(1, w), (128, 1))
    shared = dict(
        wgat=Wg, vsd=vsd, iota=iota,
        wa=np.asarray(Wa, np.float32), w1=np.asarray(W1, np.float32),
        w2=np.asarray(W2, np.float32), w3=np.asarray(W3, np.float32),
        bgat=rep(b_gat, 512), ba_r=rep(ba, 256), ga_r=rep(ga, 256),
        bna_r=rep(bna, 256), b1_r=rep(b1, 128), g1_r=rep(g1, 128),
        bn1_r=rep(bn1, 128), b2_r=rep(b2, 64), g2_r=rep(g2, 64),
        bn2_r=rep(bn2, 64), b3c=np.asarray(b3, np.float32).reshape(3, 1),
        constm05=np.full((1, N), -0.5, np.float32))
    in_maps = []
    for core in range(NC):
        offs_a = np.zeros((128, c.NKT), np.int32)
        dstm_a = np.full((128, c.NKT), -1.0, np.float32)
        lo, hi = core * c.NPC, (core + 1) * c.NPC
        m = (dst >= lo) & (dst < hi)
        s_c, d_c = src[m], dst[m] - lo
        for b in range(c.NBLK):
            mb_ = (d_c >= b * 125) & (d_c < (b + 1) * 125)
            sb_, db_ = s_c[mb_], d_c[mb_] - b * 125
            ne = len(sb_)
            assert ne <= c.EPB, f"block overflow: {ne} > {c.EPB}"
            t_idx, p_idx = np.divmod(np.arange(ne), 128)
            offs_a[p_idx, b * c.KT + t_idx] = sb_
            dstm_a[p_idx, b * c.KT + t_idx] = db_
        xT_sh = np.ascontiguousarray(x[lo:hi].T)
        dgo_a = np.zeros((128, 16), np.int64)
        for bq in range(c.NBLK):
            p = np.arange(125)
            dgo_a[0:125, bq] = (bq * 125 + p) * N + (lo + bq * 125 + p)
        in_maps.append(dict(shared, xT=xT_sh, offs=offs_a, dstm=dstm_a,
                            dgo=dgo_a.astype(np.int32)))
    return in_maps


_CACHE = {}


def kernel(**inputs) -> np.ndarray:
    cfg = Cfg()
    if "nc" not in _CACHE:
        _CACHE["nc"] = build_nc(cfg)
    nc = _CACHE["nc"]
    in_maps = host_prep(cfg, **inputs)
    res = run_bass_kernel_spmd(nc, in_maps, core_ids=list(range(NC)))
    return np.concatenate([res.results[cx]["D"] for cx in range(NC)], axis=0)


# revision 17
# speedup vs baseline: 9306.1747x; 9306.1747x over previous
"""GAT (2 heads x 256) + 4-layer MLP with LayerNorms + pairwise-distance output,
distributed over 8 Trainium2 NeuronCores via Bass/Tile.

Strategy (graph partition over destination nodes):
  - Stage A: each core computes its shard of the node table
    [h0 |1|0| h1 |1|0| a_src] = x @ [W_gat | v_src] (f32r-rounded), plus a
    local a_dst table; the node table is AllGather'ed.
  - GAT: edges pre-sorted by dst into blocks of 125 dsts (padded); per
    128-edge tile: indirect-DMA row gather; per-edge a_dst extracted by
    one-hot mask + reduce; w = exp(leaky_relu(a_src+a_dst, 0.2)); weighted
    one-hot ATw; U|z = ATw^T @ [h|1] on PE (softmax denominator via the
    embedded ones column); normalize by 1/z.
  - MLP fused per block (PE transpose + f32r matmul + LayerNorm on DVE),
    emitting transposed embeddings h3T [3, nodes/core]; AllGather.
  - cdist: d^2 = u.v with u=[h,1,sq_i], v=[-2h,sq_j,1]; both operands are
    staged as (-1/2)x scaled rows so each operand is produced by ONE
    uniform base-0 DVE scale op (partition-alignment rule); the epilogue
    computes sqrt(max(d2,0)). Diagonal zeroed exactly by
    an indirect-DMA scatter. Output row-sharded [N/8, N] per core.
"""
import sys
import os

for _p in ("/opt/trn_rl_repo", "/root/.axon_site/_ro/trn_rl_repo"):
    if os.path.isdir(_p) and _p not in sys.path:
        sys.path.append(_p)

import numpy as np
from concourse import bass, bacc, tile, mybir
from concourse.bass_utils import run_bass_kernel_spmd
from concourse.masks import make_identity

F32 = mybir.dt.float32
F32R = mybir.dt.float32r
I32 = mybir.dt.int32
AF = mybir.ActivationFunctionType
OP = mybir.AluOpType
AX = mybir.AxisListType

NC = 8
LN_EPS = 1e-5


class Cfg:
    def __init__(self, N=10000, EPB=4480, FD=512):
        self.N = N
        self.FD = FD
        self.NPC = N // NC
        self.BLK = 125
        self.NBLK = self.NPC // self.BLK
        self.EPB = EPB
        self.KT = EPB // 128
        self.NKT = self.NBLK * self.KT
        self.TW = 528
        assert self.NPC % self.BLK == 0 and EPB % 128 == 0
        npc = self.NPC
        c0 = (npc // 3 + 1) & ~1
        c1 = ((npc - c0) // 2 + 1) & ~1
        c2 = npc - c0 - c1
        self.CH = [c0, c1, c2]
        assert all(cw % 2 == 0 and cw <= 512 for cw in self.CH)
        assert sum(self.CH) == npc


def _ap(t_ap, offset, pattern):
    return bass.AP(t_ap.tensor, offset, pattern)


def build_nc(cfg: Cfg):
    c = cfg
    nc = bacc.Bacc("TRN2", target_bir_lowering=False, debug=False,
                   enable_asserts=True, num_devices=NC)
    xT = nc.dram_tensor("xT", [c.FD, c.NPC], F32, kind="ExternalInput").ap()
    offs = nc.dram_tensor("offs", [128, c.NKT], I32, kind="ExternalInput").ap()
    dstm = nc.dram_tensor("dstm", [128, c.NKT], F32, kind="ExternalInput").ap()
    iota = nc.dram_tensor("iota", [128, 128], F32, kind="ExternalInput").ap()
    wgat = nc.dram_tensor("wgat", [c.FD, 512], F32, kind="ExternalInput").ap()
    vsd = nc.dram_tensor("vsd", [c.FD, 4], F32, kind="ExternalInput").ap()
    wa = nc.dram_tensor("wa", [512, 256], F32, kind="ExternalInput").ap()
    w1 = nc.dram_tensor("w1", [256, 128], F32, kind="ExternalInput").ap()
    w2 = nc.dram_tensor("w2", [128, 64], F32, kind="ExternalInput").ap()
    w3 = nc.dram_tensor("w3", [64, 3], F32, kind="ExternalInput").ap()
    bgat = nc.dram_tensor("bgat", [128, 512], F32, kind="ExternalInput").ap()
    ba_r = nc.dram_tensor("ba_r", [128, 256], F32, kind="ExternalInput").ap()
    ga_r = nc.dram_tensor("ga_r", [128, 256], F32, kind="ExternalInput").ap()
    bna_r = nc.dram_tensor("bna_r", [128, 256], F32, kind="ExternalInput").ap()
    b1_r = nc.dram_tensor("b1_r", [128, 128], F32, kind="ExternalInput").ap()
    g1_r = nc.dram_tensor("g1_r", [128, 128], F32, kind="ExternalInput").ap()
    bn1_r = nc.dram_tensor("bn1_r", [128, 128], F32, kind="ExternalInput").ap()
    b2_r = nc.dram_tensor("b2_r", [128, 64], F32, kind="ExternalInput").ap()
    g2_r = nc.dram_tensor("g2_r", [128, 64], F32, kind="ExternalInput").ap()
    bn2_r = nc.dram_tensor("bn2_r", [128, 64], F32, kind="ExternalInput").ap()
    b3c = nc.dram_tensor("b3c", [3, 1], F32, kind="ExternalInput").ap()
    dgo = nc.dram_tensor("dgo", [128, 16], I32, kind="ExternalInput").ap()
    constm05 = nc.dram_tensor("constm05", [1, c.N], F32,
                              kind="ExternalInput").ap()
    D = nc.dram_tensor("D", [c.NPC, c.N], F32, kind="ExternalOutput").ap()
    DBG = os.environ.get("KDBG") == "1"
    TREP = int(os.environ.get("KTIME", "0"))
    if DBG:
        dbg_h3 = nc.dram_tensor("dbg_h3", [3 * NC, c.NPC], F32,
                                kind="ExternalOutput").ap()
        dbg_tbl = nc.dram_tensor("dbg_tbl", [128, c.TW], F32,
                                 kind="ExternalOutput").ap()
        dbg_u = nc.dram_tensor("dbg_u", [125, 516], F32,
                               kind="ExternalOutput").ap()
        dbg_ad = nc.dram_tensor("dbg_ad", [128, 8], F32,
                                kind="ExternalOutput").ap()
    tbl_sh = nc.dram_tensor("tbl_sh", [c.NPC, c.TW], F32, kind="Internal").ap()
    tbl_full = nc.dram_tensor("tbl_full", [c.N, c.TW], F32,
                              kind="Internal", addr_space="Shared").ap()
    attnD = nc.dram_tensor("attnD", [c.NPC, 2], F32, kind="Internal").ap()
    h3T_sh = nc.dram_tensor("h3T_sh", [3, c.NPC], F32, kind="Internal").ap()
    h3T_full = nc.dram_tensor("h3T_full", [3 * NC, c.NPC], F32,
                              kind="Internal", addr_space="Shared").ap()
    sqd = nc.dram_tensor("sqd", [1, c.N], F32, kind="Internal").ap()
    usqd = nc.dram_tensor("usqd", [1, c.NPC], F32, kind="Internal").ap()

    def load_w_r(pool, apx, K, Nn, tag):
        tiles = []
        for k in range(0, K, 128):
            tf = pool.tile([128, Nn], F32, tag=f"{tag}f")
            nc.sync.dma_start(out=tf[:], in_=apx[k:k + 128, :])
            tr = pool.tile([128, Nn], F32R, tag=f"{tag}r{k}")
            nc.vector.tensor_copy(tr[:], tf[:])
            tiles.append(tr)
        return tiles

    with tile.TileContext(nc) as tc:
        with tc.tile_pool(name="const", bufs=1) as cp:
            # ---------- constants ----------
            iota_sb = cp.tile([128, 128], F32)
            nc.sync.dma_start(out=iota_sb[:], in_=iota[:])
            ident = cp.tile([128, 128], F32)
            make_identity(nc, ident[:])
            c10 = cp.tile([128, 2], F32)
            nc.vector.memset(c10[:, 0:1], 1.0)
            nc.vector.memset(c10[:, 1:2], 0.0)
            zpad = cp.tile([128, 12], F32)
            nc.vector.memset(zpad[:], 0.0)
            ones31 = cp.tile([3, 1], F32)
            nc.vector.memset(ones31[:], 1.0)
            eps_sb = cp.tile([128, 1], F32)
            nc.vector.memset(eps_sb[:], LN_EPS)
            b3_sb = cp.tile([3, 1], F32)
            nc.sync.dma_start(out=b3_sb[:], in_=b3c[:])
            dgo_sb = cp.tile([128, 16], I32)
            nc.sync.dma_start(out=dgo_sb[:], in_=dgo[:])
            reps = {}
            for nm, apx, wdt in (("bgat", bgat, 512), ("ba", ba_r, 256),
                                 ("ga", ga_r, 256), ("bna", bna_r, 256),
                                 ("b1", b1_r, 128), ("g1", g1_r, 128),
                                 ("bn1", bn1_r, 128), ("b2", b2_r, 64),
                                 ("g2", g2_r, 64), ("bn2", bn2_r, 64)):
                tl = cp.tile([128, wdt], F32, tag=f"rep_{nm}")
                nc.sync.dma_start(out=tl[:], in_=apx[:])
                reps[nm] = tl
            wa_t = load_w_r(cp, wa, 512, 256, "wa")
            w1_t = load_w_r(cp, w1, 256, 128, "w1")
            w2_t = load_w_r(cp, w2, 128, 64, "w2")
            w3_f = cp.tile([64, 3], F32)
            nc.sync.dma_start(out=w3_f[:], in_=w3[:])
            offs_sb = cp.tile([128, c.NKT], I32)
            nc.sync.dma_start(out=offs_sb[:], in_=offs[:])
            dstm_sb = cp.tile([128, c.NKT], F32)
            nc.sync.dma_start(out=dstm_sb[:], in_=dstm[:])
            h3T_sb = cp.tile([3, c.NPC], F32)

            # ---------- stage A ----------
            with tc.tile_pool(name="sa", bufs=2) as sa, \
                 tc.tile_pool(name="psa", bufs=2, space="PSUM") as psa:
                nk = c.FD // 128
                xT_r = []
                for k in range(nk):
                    xf = sa.tile([128, c.NPC], F32, tag="xf")
                    nc.sync.dma_start(out=xf[:], in_=xT[k * 128:(k + 1) * 128, :])
                    xr = sa.tile([128, c.NPC], F32R, tag=f"xr{k}")
                    nc.vector.tensor_copy(xr[:], xf[:])
                    xT_r.append(xr)
                wg_t = load_w_r(sa, wgat, c.FD, 512, "wg")
                vs_t = load_w_r(sa, vsd, c.FD, 4, "vs")
                for mt in range(c.NBLK):
                    ms = mt * 125
                    ph = psa.tile([125, 512], F32, space="PSUM", tag="ph")
                    pa = psa.tile([125, 4], F32, space="PSUM", tag="pa")
                    for k in range(nk):
                        lhs = xT_r[k][:, ms:ms + 125]
                        nc.tensor.matmul(ph[:], lhsT=lhs, rhs=wg_t[k][:],
                                         start=(k == 0), stop=(k == nk - 1))
                        nc.tensor.matmul(pa[:], lhsT=lhs, rhs=vs_t[k][:],
                                         start=(k == 0), stop=(k == nk - 1))
                    tb = sa.tile([125, c.TW], F32R, tag="tb")
                    nc.vector.tensor_copy(tb[:, 0:256], ph[:, 0:256])
                    nc.vector.tensor_copy(tb[:, 256:258], c10[0:125, :])
                    nc.vector.tensor_copy(tb[:, 258:514], ph[:, 256:512])
                    nc.vector.tensor_copy(tb[:, 514:516], c10[0:125, :])
                    nc.vector.tensor_copy(tb[:, 516:518], pa[:, 0:2])
                    nc.vector.tensor_copy(tb[:, 518:528], zpad[0:125, 0:10])
                    nc.sync.dma_start(out=tbl_sh[ms:ms + 125, :],
                                      in_=tb[:].bitcast(F32))
                    ad = sa.tile([125, 2], F32, tag="ad")
                    nc.vector.tensor_copy(ad[:], pa[:, 2:4])
                    nc.sync.dma_start(out=attnD[ms:ms + 125, :], in_=ad[:])

            nc.gpsimd.collective_compute(
                "AllGather", OP.bypass, ins=[tbl_sh[:]], outs=[tbl_full[:]],
                replica_groups=[list(range(NC))])
            tbl_full_r = tbl_full.bitcast(F32R)

            # ---------- GAT + MLP ----------
            with tc.tile_pool(name="gb", bufs=10) as gb, \
                 tc.tile_pool(name="eb", bufs=6) as eb, \
                 tc.tile_pool(name="mb", bufs=3) as mb, \
                 tc.tile_pool(name="psu", bufs=2, space="PSUM") as psu, \
                 tc.tile_pool(name="pst", bufs=2, space="PSUM") as pst:
              if True:
                import contextlib
                loop_cm = tc.For_i(0, TREP, 1) if TREP > 1 \
                    else contextlib.nullcontext()
                with loop_cm:
                  for b in range(c.NBLK):
                    adb = eb.tile([128, 250], F32, tag="adb")
                    for hh in range(2):
                        src_ap = _ap(attnD, 125 * b * 2 + hh,
                                     [[0, 128], [2, 125]])
                        nc.sync.dma_start(out=adb[:, hh * 125:(hh + 1) * 125],
                                          in_=src_ap)
                    U0 = psu.tile([125, 258], F32, space="PSUM", tag="U0")
                    U1 = psu.tile([125, 258], F32, space="PSUM", tag="U1")
                    for t in range(c.KT):
                        j = b * c.KT + t
                        G = gb.tile([128, c.TW], F32R, tag="G")
                        nc.gpsimd.indirect_dma_start(
                            out=G[:], out_offset=None, in_=tbl_full_r,
                            in_offset=bass.IndirectOffsetOnAxis(
                                ap=offs_sb[:, j:j + 1], axis=0))
                        ind01 = eb.tile([128, 125], F32, tag="ind01")
                        nc.vector.tensor_scalar(
                            out=ind01[:], in0=iota_sb[:, 0:125],
                            scalar1=dstm_sb[:, j:j + 1], scalar2=None,
                            op0=OP.is_equal)
                        msk = eb.tile([128, 250], F32, tag="msk")
                        i01 = ind01[:]
                        ind2 = _ap(i01, i01.offset,
                                   [i01.ap[0], [0, 2]] + list(i01.ap[1:]))
                        nc.vector.tensor_tensor(out=msk[:], in0=adb[:],
                                                in1=ind2, op=OP.mult)
                        ad_e = eb.tile([128, 2], F32, tag="ad_e")
                        mk = msk[:]
                        mk3 = _ap(mk, mk.offset, [mk.ap[0], [125, 2], [1, 125]])
                        nc.vector.reduce_sum(out=ad_e[:], in_=mk3, axis=AX.X)
                        l_e = eb.tile([128, 2], F32, tag="l_e")
                        nc.vector.tensor_tensor(
                            out=l_e[:], in0=ad_e[:],
                            in1=G[:, 516:518].bitcast(F32), op=OP.add)
                        lk = eb.tile([128, 2], F32, tag="lk")
                        nc.scalar.activation(lk[:], l_e[:], AF.Prelu, alpha=0.2)
                        w_e = eb.tile([128, 2], F32, tag="w_e")
                        nc.scalar.activation(w_e[:], lk[:], AF.Exp)
                        atw0 = eb.tile([128, 125], F32R, tag="atw0")
                        nc.vector.tensor_scalar(
                            out=atw0[:], in0=iota_sb[:, 0:125],
                            scalar1=dstm_sb[:, j:j + 1], scalar2=w_e[:, 0:1],
                            op0=OP.is_equal, op1=OP.mult)
                        atw1 = eb.tile([128, 125], F32R, tag="atw1")
                        nc.vector.tensor_scalar(
                            out=atw1[:], in0=iota_sb[:, 0:125],
                            scalar1=dstm_sb[:, j:j + 1], scalar2=w_e[:, 1:2],
                            op0=OP.is_equal, op1=OP.mult)
                        if DBG and b == 0 and t == 0:
                            dad = eb.tile([128, 8], F32, tag="dbgad")
                            nc.vector.tensor_copy(dad[:, 0:2], ad_e[:])
                            nc.vector.tensor_copy(dad[:, 2:4], l_e[:])
                            nc.vector.tensor_copy(dad[:, 4:6], w_e[:])
                            nc.vector.tensor_copy(
                                dad[:, 6:8], G[:, 516:518].bitcast(F32))
                            nc.sync.dma_start(out=dbg_ad[:], in_=dad[:])
                        st, sp = (t == 0), (t == c.KT - 1)
                        nc.tensor.matmul(U0[:], lhsT=atw0[:], rhs=G[:, 0:258],
                                         start=st, stop=sp)
                        nc.tensor.matmul(U1[:], lhsT=atw1[:], rhs=G[:, 258:516],
                                         start=st, stop=sp)
                    if DBG and b == 0:
                        du = mb.tile([125, 516], F32, tag="dbgu")
                        nc.vector.tensor_copy(du[:, 0:258], U0[:])
                        nc.vector.tensor_copy(du[:, 258:516], U1[:])
                        nc.sync.dma_start(out=dbg_u[:], in_=du[:])
                    rz = eb.tile([125, 2], F32, tag="rz")
                    nc.vector.reciprocal(rz[:, 0:1], U0[:, 256:257])
                    nc.vector.reciprocal(rz[:, 1:2], U1[:, 256:257])
                    a0 = mb.tile([125, 512], F32, tag="a0")
                    nc.vector.tensor_scalar(out=a0[:, 0:256], in0=U0[:, 0:256],
                                            scalar1=rz[:, 0:1], scalar2=None,
                                            op0=OP.mult)
                    nc.vector.tensor_scalar(out=a0[:, 256:512], in0=U1[:, 0:256],
                                            scalar1=rz[:, 1:2], scalar2=None,
                                            op0=OP.mult)
                    nc.vector.tensor_tensor(out=a0[:], in0=a0[:],
                                            in1=reps["bgat"][0:125, :], op=OP.add)
                    nc.vector.tensor_scalar(out=a0[:], in0=a0[:], scalar1=0.0,
                                            scalar2=None, op0=OP.max)

                    def dense(act, K, Nn, wt):
                        ps = pst.tile([125, 256], F32, space="PSUM", tag="mm")
                        nkk = K // 128
                        for kk in range(nkk):
                            tp = pst.tile([128, 125], F32, space="PSUM",
                                          tag="tp")
                            nc.tensor.transpose(
                                out=tp[:], in_=act[:, kk * 128:(kk + 1) * 128],
                                identity=ident[0:125, 0:125])
                            aT = mb.tile([128, 125], F32R, tag="aT")
                            nc.vector.tensor_copy(aT[:], tp[:])
                            nc.tensor.matmul(ps[:, 0:Nn], lhsT=aT[:],
                                             rhs=wt[kk][:],
                                             start=(kk == 0), stop=(kk == nkk - 1))
                        return ps

                    def ln(ps, Nn, bias, gam, bet, tag):
                        t_ = mb.tile([125, Nn], F32, tag=f"t{tag}")
                        nc.vector.tensor_tensor(out=t_[:], in0=ps[:, 0:Nn],
                                                in1=bias[0:125, :], op=OP.add)
                        s_ = eb.tile([125, 1], F32, tag=f"s{tag}")
                        nc.vector.reduce_sum(out=s_[:], in_=t_[:], axis=AX.X)
                        nc.vector.tensor_scalar(out=s_[:], in0=s_[:],
                                                scalar1=1.0 / Nn, scalar2=None,
                                                op0=OP.mult)
                        xc = mb.tile([125, Nn], F32, tag=f"xc{tag}")
                        nc.vector.tensor_scalar(out=xc[:], in0=t_[:],
                                                scalar1=s_[:], scalar2=None,
                                                op0=OP.subtract)
                        x2 = mb.tile([125, Nn], F32, tag=f"x2{tag}")
                        nc.vector.tensor_tensor(out=x2[:], in0=xc[:], in1=xc[:],
                                                op=OP.mult)
                        ss = eb.tile([125, 1], F32, tag=f"ss{tag}")
                        nc.vector.reduce_sum(out=ss[:], in_=x2[:], axis=AX.X)
                        sd = eb.tile([125, 1], F32, tag=f"sd{tag}")
                        nc.scalar.activation(sd[:], ss[:], AF.Sqrt,
                                             bias=eps_sb[0:125, 0:1],
                                             scale=1.0 / Nn)
                        rstd = eb.tile([125, 1], F32, tag=f"rstd{tag}")
                        nc.vector.reciprocal(rstd[:], sd[:])
                        y = mb.tile([125, Nn], F32, tag=f"y{tag}")
                        nc.vector.tensor_scalar(out=y[:], in0=xc[:],
                                                scalar1=rstd[:], scalar2=None,
                                                op0=OP.mult)
                        nc.vector.tensor_tensor(out=y[:], in0=y[:],
                                                in1=gam[0:125, :], op=OP.mult)
                        nc.vector.tensor_tensor(out=y[:], in0=y[:],
                                                in1=bet[0:125, :], op=OP.add)
                        return y

                    psa_ = dense(a0, 512, 256, wa_t)
                    ya = ln(psa_, 256, reps["ba"], reps["ga"], reps["bna"], "a")
                    nc.vector.tensor_scalar(out=ya[:], in0=ya[:], scalar1=0.0,
                                            scalar2=None, op0=OP.max)
                    ps1 = dense(ya, 256, 128, w1_t)
                    y1 = ln(ps1, 128, reps["b1"], reps["g1"], reps["bn1"], "b")
                    nc.vector.tensor_scalar(out=y1[:], in0=y1[:], scalar1=0.0,
                                            scalar2=None, op0=OP.max)
                    nc.scalar.activation(y1[:], y1[:], AF.Tanh)
                    ps2 = dense(y1, 128, 64, w2_t)
                    y2 = ln(ps2, 64, reps["b2"], reps["g2"], reps["bn2"], "c")
                    nc.vector.tensor_scalar(out=y2[:], in0=y2[:], scalar1=0.0,
                                            scalar2=None, op0=OP.max)
                    tp3 = pst.tile([128, 125], F32, space="PSUM", tag="tp")
                    nc.tensor.transpose(out=tp3[0:64, :], in_=y2[:],
                                        identity=ident[0:125, 0:125])
                    a3T = mb.tile([64, 125], F32, tag="a3T")
                    nc.vector.tensor_copy(a3T[:], tp3[0:64, :])
                    ph3 = pst.tile([125, 256], F32, space="PSUM", tag="mm")
                    nc.tensor.matmul(ph3[0:3, 0:125], lhsT=w3_f[:], rhs=a3T[:],
                                     start=True, stop=True)
                    nc.vector.tensor_scalar(out=h3T_sb[:, b * 125:(b + 1) * 125],
                                            in0=ph3[0:3, 0:125],
                                            scalar1=b3_sb[:, 0:1],
                                            scalar2=None, op0=OP.add)
            nc.sync.dma_start(out=h3T_sh[:], in_=h3T_sb[:])

            nc.gpsimd.collective_compute(
                "AllGather", OP.bypass, ins=[h3T_sh[:]], outs=[h3T_full[:]],
                replica_groups=[list(range(NC))])

            if DBG:
                dtmp = cp.tile([24, c.NPC], F32, tag="dbg1")
                nc.sync.dma_start(out=dtmp[:], in_=h3T_full[:])
                nc.sync.dma_start(out=dbg_h3[:], in_=dtmp[:])
                dtb = cp.tile([128, c.TW], F32, tag="dbg2")
                nc.sync.dma_start(out=dtb[:], in_=tbl_full[0:128, :])
                nc.sync.dma_start(out=dbg_tbl[:], in_=dtb[:])
            # ---------- cdist ----------
            with tc.tile_pool(name="cdb", bufs=1) as cdb, \
                 tc.tile_pool(name="ob", bufs=(1 if DBG else 2)) as ob, \
                 tc.tile_pool(name="psq", bufs=2, space="PSUM") as psq, \
                 tc.tile_pool(name="psd", bufs=6, space="PSUM") as psd:
                Vr = cdb.tile([5, c.N], F32R)
                Ur = cdb.tile([5, c.NPC], F32R)
                with tc.tile_pool(name="cbu", bufs=1) as cbu:
                    # Vf5 = [h | -sq_j/2 | -1/2] ; Vr = -2*Vf5 (f32r)
                    Vf5 = cbu.tile([5, c.N], F32, tag="Vf5")
                    for r in range(NC):
                        nc.sync.dma_start(
                            out=Vf5[0:3, r * c.NPC:(r + 1) * c.NPC],
                            in_=h3T_full[3 * r:3 * r + 3, :])
                    # sq of the f32r-rounded h (same bits the matmul sees)
                    otmp = cbu.tile([3, c.NPC], F32, tag="otmp")
                    hreg = cbu.tile([3, c.NPC], F32R, tag="hreg")
                    for r in range(NC):
                        base = r * c.NPC
                        nc.vector.tensor_copy(hreg[:],
                                              Vf5[0:3, base:base + c.NPC])
                        hsl = hreg[:].bitcast(F32)
                        nc.vector.tensor_tensor(out=otmp[:], in0=hsl,
                                                in1=hsl, op=OP.mult)
                        for ci, cw in enumerate(c.CH):
                            off = sum(c.CH[:ci])
                            pq = psq.tile([1, 512], F32, space="PSUM", tag="pq")
                            nc.tensor.matmul(pq[:, 0:cw], lhsT=ones31[:],
                                             rhs=otmp[:, off:off + cw],
                                             start=True, stop=True)
                            sqst = cbu.tile([1, 512], F32, tag="sqst")
                            nc.vector.tensor_scalar(
                                out=sqst[:, 0:cw], in0=pq[0:1, 0:cw],
                                scalar1=-0.5, scalar2=None, op0=OP.mult)
                            nc.sync.dma_start(
                                out=sqd[:, base + off:base + off + cw],
                                in_=sqst[:, 0:cw])
                    nc.sync.dma_start(out=Vf5[3:4, :], in_=sqd[:])
                    nc.sync.dma_start(out=Vf5[4:5, :], in_=constm05[:])
                    nc.vector.tensor_scalar(out=Vr[:], in0=Vf5[:],
                                            scalar1=-2.0, scalar2=None,
                                            op0=OP.mult)
                    # Uf5 = [-h/2 | -1/2 | -sq_i/2] ; Ur = -2*Uf5 (f32r)
                    Uf5 = cbu.tile([5, c.NPC], F32, tag="Uf5")
                    nc.sync.dma_start(out=Uf5[0:3, :], in_=h3T_sh[:])
                    nc.vector.tensor_scalar(out=Uf5[0:3, :], in0=Uf5[0:3, :],
                                            scalar1=-0.5, scalar2=None,
                                            op0=OP.mult)
                    nc.sync.dma_start(out=Uf5[3:4, :],
                                      in_=constm05[:, 0:c.NPC])
                    # own sq from the rounded own h (same bits as V rows)
                    ur3 = cbu.tile([3, c.NPC], F32R, tag="ur3")
                    nc.vector.tensor_scalar(out=ur3[:], in0=Uf5[0:3, :],
                                            scalar1=-2.0, scalar2=None,
                                            op0=OP.mult)
                    u3f = ur3[:].bitcast(F32)
                    nc.vector.tensor_tensor(out=otmp[:], in0=u3f, in1=u3f,
                                            op=OP.mult)
                    for ci, cw in enumerate(c.CH):
                        off = sum(c.CH[:ci])
                        pq = psq.tile([1, 512], F32, space="PSUM", tag="pq")
                        nc.tensor.matmul(pq[:, 0:cw], lhsT=ones31[:],
                                         rhs=otmp[:, off:off + cw],
                                         start=True, stop=True)
                        sqst = cbu.tile([1, 512], F32, tag="sqst")
                        nc.vector.tensor_scalar(
                            out=sqst[:, 0:cw], in0=pq[0:1, 0:cw],
                            scalar1=-0.5, scalar2=None, op0=OP.mult)
                        nc.sync.dma_start(out=usqd[:, off:off + cw],
                                          in_=sqst[:, 0:cw])
                    nc.sync.dma_start(out=Uf5[4:5, :], in_=usqd[:])
                    nc.vector.tensor_scalar(out=Ur[:], in0=Uf5[:],
                                            scalar1=-2.0, scalar2=None,
                                            op0=OP.mult)
                for mt in range(c.NBLK):
                    ms = mt * 125
                    outb = ob.tile([125, c.N], F32, tag="outb")
                    for r in range(NC):
                        base = r * c.NPC
                        for ci, cw in enumerate(c.CH):
                            off = base + sum(c.CH[:ci])
                            pd = psd.tile([125, 512], F32, space="PSUM",
                                          tag="pd")
                            nc.tensor.matmul(pd[:, 0:cw],
                                             lhsT=Ur[:, ms:ms + 125],
                                             rhs=Vr[:, off:off + cw],
                                             start=True, stop=True)
                            nc.vector.tensor_scalar(
                                out=outb[:, off:off + cw], in0=pd[:, 0:cw],
                                scalar1=0.0, scalar2=None, op0=OP.max)
                            nc.scalar.activation(outb[:, off:off + cw],
                                                 outb[:, off:off + cw],
                                                 AF.Sqrt)
                    nc.sync.dma_start(out=D[ms:ms + 125, :], in_=outb[:])
                    nc.gpsimd.indirect_dma_start(
                        out=D[:], in_=c10[0:125, 1:2],
                        out_offset=bass.IndirectOffsetOnAxis(
                            ap=dgo_sb[0:125, mt:mt + 1], axis=1),
                        in_offset=None)
    nc.compile()
    return nc


def host_prep(cfg: Cfg, x, edge_index, W_gat, att_src, att_dst, b_gat,
              Wa, ba, ga, bna, W1, b1, g1, bn1, W2, b2, g2, bn2, W3, b3):
    c = cfg
    N = c.N
    x = np.asarray(x, np.float32)
    ei = np.asarray(edge_index).astype(np.int64)
    src = np.concatenate([ei[0], np.arange(N, dtype=np.int64)])
    dst = np.concatenate([ei[1], np.arange(N, dtype=np.int64)])
    order = np.argsort(dst, kind="stable")
    src, dst = src[order], dst[order]
    Wg = np.asarray(W_gat, np.float32)
    a_s = np.asarray(att_src, np.float32)
    a_d = np.asarray(att_dst, np.float32)
    Wg3 = Wg.reshape(c.FD, 2, 256)
    v_src = np.einsum("khc,hc->kh", Wg3, a_s).astype(np.float32)
    v_dst = np.einsum("khc,hc->kh", Wg3, a_d).astype(np.float32)
    vsd = np.concatenate([v_src, v_dst], axis=1)
    iota = np.tile(np.arange(128, dtype=np.float32), (128, 1))
    rep = lambda v, w: np.tile(np.asarray(v, np.float32).reshape(1, w), (128, 1))
    shared = dict(
        wgat=Wg, vsd=vsd, iota=iota,
        wa=np.asarray(Wa, np.float32), w1=np.asarray(W1, np.float32),
        w2=np.asarray(W2, np.float32), w3=np.asarray(W3, np.float32),
        bgat=rep(b_gat, 512), ba_r=rep(ba, 256), ga_r=rep(ga, 256),
        bna_r=rep(bna, 256), b1_r=rep(b1, 128), g1_r=rep(g1, 128),
        bn1_r=rep(bn1, 128), b2_r=rep(b2, 64), g2_r=rep(g2, 64),
        bn2_r=rep(bn2, 64), b3c=np.asarray(b3, np.float32).reshape(3, 1),
        constm05=np.full((1, N), -0.5, np.float32))
    in_maps = []
    for core in range(NC):
        offs_a = np.zeros((128, c.NKT), np.int32)
        dstm_a = np.full((128, c.NKT), -1.0, np.float32)
        lo, hi = core * c.NPC, (core + 1) * c.NPC
        m = (dst >= lo) & (dst < hi)
        s_c, d_c = src[m], dst[m] - lo
        for b in range(c.NBLK):
            mb_ = (d_c >= b * 125) & (d_c < (b + 1) * 125)
            sb_, db_ = s_c[mb_], d_c[mb_] - b * 125
            ne = len(sb_)
            assert ne <= c.EPB, f"block overflow: {ne} > {c.EPB}"
            t_idx, p_idx = np.divmod(np.arange(ne), 128)
            offs_a[p_idx, b * c.KT + t_idx] = sb_
            dstm_a[p_idx, b * c.KT + t_idx] = db_
        xT_sh = np.ascontiguousarray(x[lo:hi].T)
        dgo_a = np.zeros((128, 16), np.int64)
        for bq in range(c.NBLK):
            p = np.arange(125)
            dgo_a[0:125, bq] = (bq * 125 + p) * N + (lo + bq * 125 + p)
        in_maps.append(dict(shared, xT=xT_sh, offs=offs_a, dstm=dstm_a,
                            dgo=dgo_a.astype(np.int32)))
    return in_maps


_CACHE = {}


def kernel(**inputs) -> np.ndarray:
    cfg = Cfg()
    if "nc" not in _CACHE:
        _CACHE["nc"] = build_nc(cfg)
    nc = _CACHE["nc"]
    in_maps = host_prep(cfg, **inputs)
    res = run_bass_kernel_spmd(nc, in_maps, core_ids=list(range(NC)))
    return np.concatenate([res.results[cx]["D"] for cx in range(NC)], axis=0)
